# revision 12
# baseline (speedup 1.0000x reference)
"""Trainium2 Bass kernel for Llama4TextExperts-style grouped MoE FFN (SwiGLU).

Full-input contract: kernel(**inputs) takes the complete unsharded tensors and
returns the full [4096, 1024] output. Internally: expert-parallel across the 8
NeuronCores — core e gets expert e's three weight matrices and that expert's
512-token group (tokens arrive pre-sorted by expert with equal group sizes).
All routing / transposition is done host-side in numpy; no collectives needed.

Per-core device program (three GEMMs, ~6.4 GFLOP):
  phase 1: gate^T = Wg^T-stationary @ x^T, up^T likewise; SwiGLU fused on
           ACT (Silu) + DVE (mul) into h^T [I, T] bf16 resident in SBUF.
  phase 2: y = h @ Wd with h^T slices stationary, Wd streaming; y lands
           untransposed in PSUM.

All DRAM parameters are host-packed partition-major so every DMA moves
2-16KB contiguous runs per partition (vs 256B for a naive [H, I] layout):
the DMA pipe hits peak rate sooner, which sets how early the PE can leave
its HAM warm-up and enter the real matmul stream.
"""

import numpy as np
import ml_dtypes

import concourse.bass as bass
import concourse.mybir as mybir
import concourse.tile as tile
from concourse.tile import add_dep_helper
from concourse import bacc
from concourse.bass_utils import run_bass_kernel_spmd

# Problem shape (hardcoded per contract)
E = 8          # experts == cores
T = 512        # tokens per expert group
H = 1024       # hidden
I = 2048       # intermediate
P = 128        # partitions
KT = H // P    # 8  k-tiles over hidden
IT = I // P    # 16 i-tiles over intermediate
WB = 4         # i-blocks of 512 columns for gate/up weight streaming
MT = T // P    # 4  token tiles
N_WARM = 5     # HAM warm-up matmuls before the first real chain (512 wide)

BF16 = mybir.dt.bfloat16
F32 = mybir.dt.float32

_compiled = None  # cached across calls


def _build():
    nc = bacc.Bacc(None)
    # Host-packed inputs: every tensor is [128 partitions, N] with the
    # partition's data contiguous in DRAM.
    xp_d = nc.declare_dram_parameter("xp", [P, KT * T], BF16, isOutput=False)
    wg0_d = nc.declare_dram_parameter("wg0", [P, KT * 128], BF16, isOutput=False)
    wg1_d = nc.declare_dram_parameter("wg1", [P, KT * 128], BF16, isOutput=False)
    wg23_d = nc.declare_dram_parameter("wg23", [P, KT * 256], BF16, isOutput=False)
    wu0_d = nc.declare_dram_parameter("wu0", [P, KT * 128], BF16, isOutput=False)
    wu1_d = nc.declare_dram_parameter("wu1", [P, KT * 128], BF16, isOutput=False)
    wu23_d = nc.declare_dram_parameter("wu23", [P, KT * 256], BF16, isOutput=False)
    wgb_d = [
        nc.declare_dram_parameter(f"wgb{b}", [P, KT * 512], BF16, isOutput=False)
        for b in range(1, WB)
    ]
    wub_d = [
        nc.declare_dram_parameter(f"wub{b}", [P, KT * 512], BF16, isOutput=False)
        for b in range(1, WB)
    ]
    wd0_d = nc.declare_dram_parameter("wd0", [P, 8 * H], BF16, isOutput=False)
    wd1_d = nc.declare_dram_parameter("wd1", [P, 8 * H], BF16, isOutput=False)
    y_d = nc.declare_dram_parameter("y", [T, H], F32, isOutput=True)

    with tile.TileContext(nc) as tc:
        with (
            tc.tile_pool(name="xpool", bufs=1) as xpool,
            tc.tile_pool(name="wdpool", bufs=1) as wdpool,
            tc.tile_pool(name="hpool", bufs=1) as hpool,
            tc.tile_pool(name="wpool", bufs=3) as wpool,
            tc.tile_pool(name="spool", bufs=3) as spool,
            tc.tile_pool(name="psum", bufs=2, space="PSUM") as psum,
        ):
            # --- HAM warm-up -------------------------------------------------
            # The PE clock sits at 1.2GHz until ~3.4us of sustained busy time,
            # and RE-throttles if an activity window (~3.4us) passes mostly
            # idle. Warm-up matmuls run before the first weights land, and
            # fill_mm() dummies are woven between the first real chains so a
            # late DMA never leaves the PE idle long enough to re-throttle.
            # All dummies form one accumulation chain into a PSUM bank that
            # the down-projection won't need until much later.
            warm = xpool.tile([P, 512], BF16, tag="warm", name="warm")
            nc.vector.memset(warm[:], 0.0)
            pwarm = psum.tile([P, 512], F32, tag="py0", name="pwarm")
            fill_state = {"n": 0}

            def fill_mm(n):
                for _ in range(n):
                    nc.tensor.matmul(
                        pwarm[:], warm[:, 0:128], warm[:],
                        start=(fill_state["n"] == 0), stop=False,
                    )
                    fill_state["n"] += 1

            def fill_done():
                nc.tensor.matmul(
                    pwarm[:], warm[:, 0:128], warm[:],
                    start=False, stop=True,
                )

            fill_mm(N_WARM)

            # --- DMA issue schedule -----------------------------------------
            # Four engines issue in parallel (one instruction ~0.7us each);
            # each engine's queue is FIFO so per-engine order = priority.
            # Wave 1 (ungated): the critical set for the first it-blocks —
            # x (reused by every chain) + it0/it1 gate/up weights.
            # Later stages are gated on earlier DMA completions (lookahead
            # gating) so they don't steal round-robin bandwidth early.
            xt = xpool.tile([P, KT * T], BF16, tag="x", name="xt")
            wg_it0 = xpool.tile([P, KT * 128], BF16, tag="wg_it0")
            wg_it1 = xpool.tile([P, KT * 128], BF16, tag="wg_it1")
            wu_it0 = xpool.tile([P, KT * 128], BF16, tag="wu_it0")
            wu_it1 = xpool.tile([P, KT * 128], BF16, tag="wu_it1")
            wg_b0r = xpool.tile([P, KT * 256], BF16, tag="wg_b0r")
            wu_b0r = xpool.tile([P, KT * 256], BF16, tag="wu_b0r")
            hT_sb = hpool.tile([P, IT * T], BF16)
            wd_sb = wdpool.tile([P, IT * H], BF16)

            # Early DMAs are spread over the three issue queues (sync,
            # scalar, gpsimd) with roughly equal byte loads, each queue's
            # FIFO ordered by consumption deadline. The weight stream after
            # wave 1 lives on sync's queue alone: FIFO = strict delivery
            # priority, and a single queue sustains the full ~350GB/s once
            # the pipe clears. Staged gating (two-stage lookahead) keeps
            # ~2 stages of weight DMAs in flight so the queue head isn't
            # fighting many outstanding descriptors for round-robin
            # bandwidth.
            d0g = nc.sync.dma_start(wg_it0[:], wg0_d[:])
            nc.scalar.dma_start(xt[:, 0:2 * T], xp_d[:, 0:2 * T])
            nc.gpsimd.dma_start(xt[:, 2 * T:4 * T], xp_d[:, 2 * T:4 * T])
            d0u = nc.sync.dma_start(wu_it0[:], wu0_d[:])
            nc.scalar.dma_start(wg_it1[:], wg1_d[:])
            d_A = nc.gpsimd.dma_start(wu_it1[:], wu1_d[:])
            nc.sync.dma_start(xt[:, 4 * T:6 * T], xp_d[:, 4 * T:6 * T])
            nc.sync.dma_start(xt[:, 6 * T:8 * T], xp_d[:, 6 * T:8 * T])

            def gated(dma_call, dep):
                add_dep_helper(dma_call.ins, dep.ins, reason="dma staging throttle")
                return dma_call

            # it2/it3 of block 0, gated on the it0-g load (lookahead gating
            # keeps the pipe from draining dry between stages)
            gated(nc.sync.dma_start(wg_b0r[:], wg23_d[:]), d0g)
            d_B = gated(nc.sync.dma_start(wu_b0r[:], wu23_d[:]), d0g)
            # blocks 1..3 with two-stage lookahead gating
            wgf, wuf = {}, {}
            d_stage = [d_A, d_B]
            for wb in range(1, WB):
                wgb = wpool.tile([P, KT * 512], BF16, tag="wgf")
                gated(nc.sync.dma_start(wgb[:], wgb_d[wb - 1][:]),
                      d_stage[wb - 1])
                wgf[wb] = wgb
                wub = wpool.tile([P, KT * 512], BF16, tag="wuf")
                d_stage.append(
                    gated(nc.sync.dma_start(wub[:], wub_d[wb - 1][:]),
                          d_stage[wb - 1])
                )
                wuf[wb] = wub
            # Wd halves, chained behind the gate/up weight stream
            gated(nc.sync.dma_start(wd_sb[:, 0:8 * H], wd0_d[:]), d_stage[2])
            gated(nc.sync.dma_start(wd_sb[:, 8 * H:16 * H], wd1_d[:]), d_stage[3])

            def xk(kt):
                return xt[:, kt * T:(kt + 1) * T]

            def gu_slice(wb, itl, which):
                if wb == 0:
                    if itl < 2:
                        t = (wg_it0, wg_it1)[itl] if which == "g" else \
                            (wu_it0, wu_it1)[itl]
                        return lambda kt: t[:, kt * 128:(kt + 1) * 128]
                    t = wg_b0r if which == "g" else wu_b0r
                    base = (itl - 2) * 128
                    return lambda kt: t[:, kt * 256 + base:kt * 256 + base + 128]
                t = wgf[wb] if which == "g" else wuf[wb]
                base = itl * 128
                return lambda kt: t[:, kt * 512 + base:kt * 512 + base + 128]

            # --- gate/up + SwiGLU -------------------------------------------
            for wb in range(WB):
                for itl in range(4):
                    it = wb * 4 + itl
                    gsl = gu_slice(wb, itl, "g")
                    usl = gu_slice(wb, itl, "u")
                    pg = psum.tile([P, T], F32, tag="pg")
                    pu = psum.tile([P, T], F32, tag="pu")
                    for kt in range(KT):
                        nc.tensor.matmul(
                            pg[:], gsl(kt), xk(kt),
                            start=(kt == 0), stop=(kt == KT - 1),
                        )
                        # dummies bridge the gaps while x/weight DMAs are
                        # still ramping, keeping the activity monitor fed
                        if it == 0 and kt in (1, 3):
                            fill_mm(2)
                        elif it == 0 and kt == 5:
                            fill_mm(1)
                    if it == 0:
                        fill_done()
                    for kt in range(KT):
                        nc.tensor.matmul(
                            pu[:], usl(kt), xk(kt),
                            start=(kt == 0), stop=(kt == KT - 1),
                        )
                    sg = spool.tile([P, T], F32)
                    nc.scalar.activation(
                        sg[:], pg[:], mybir.ActivationFunctionType.Silu
                    )
                    nc.vector.tensor_mul(
                        hT_sb[:, it * T:(it + 1) * T], sg[:], pu[:]
                    )

            # --- down projection --------------------------------------------
            def wd_sl(it, c0, c1):
                return wd_sb[:, it * H + c0:it * H + c1]

            def h_sl(it, ms):
                return hT_sb[:, it * T + ms * P:it * T + (ms + 1) * P]

            for mt in range(MT):
                ms = slice(mt * P, (mt + 1) * P)
                if mt < MT - 1:
                    py0 = psum.tile([P, 512], F32, tag="py0")
                    py1 = psum.tile([P, 512], F32, tag="py1")
                    for it in range(IT):
                        lhsT = h_sl(it, mt)
                        nc.tensor.matmul(
                            py0[:], lhsT, wd_sl(it, 0, 512),
                            start=(it == 0), stop=(it == IT - 1),
                        )
                        nc.tensor.matmul(
                            py1[:], lhsT, wd_sl(it, 512, 1024),
                            start=(it == 0), stop=(it == IT - 1),
                        )
                    y0 = spool.tile([P, 512], F32, tag="y0")
                    nc.scalar.copy(y0[:], py0[:])
                    nc.sync.dma_start(y_d[ms, 0:512], y0[:])
                    y1 = spool.tile([P, 512], F32, tag="y1")
                    nc.vector.tensor_copy(y1[:], py1[:])
                    nc.gpsimd.dma_start(y_d[ms, 512:1024], y1[:])
                else:
                    # Last token tile: run chains back-to-back (not
                    # interleaved) with shrinking widths 512/256/128/128 so
                    # after the very last matmul only a [128,128] copy + DMA
                    # is exposed, and earlier copies/DMAs overlap later
                    # matmul chains.
                    py0 = psum.tile([P, 512], F32, tag="py0")
                    for it in range(IT):
                        nc.tensor.matmul(
                            py0[:], h_sl(it, mt), wd_sl(it, 0, 512),
                            start=(it == 0), stop=(it == IT - 1),
                        )
                    y0 = spool.tile([P, 512], F32, tag="y0")
                    nc.scalar.copy(y0[:], py0[:])
                    nc.sync.dma_start(y_d[ms, 0:512], y0[:])
                    py1 = psum.tile([P, 256], F32, tag="py1")
                    for it in range(IT):
                        nc.tensor.matmul(
                            py1[:], h_sl(it, mt), wd_sl(it, 512, 768),
                            start=(it == 0), stop=(it == IT - 1),
                        )
                    y1 = spool.tile([P, 256], F32, tag="y1")
                    nc.vector.tensor_copy(y1[:], py1[:])
                    nc.gpsimd.dma_start(y_d[ms, 512:768], y1[:])
                    py2 = psum.tile([P, 128], F32, tag="pu", name="py2")
                    for it in range(IT):
                        nc.tensor.matmul(
                            py2[:], h_sl(it, mt), wd_sl(it, 768, 896),
                            start=(it == 0), stop=(it == IT - 1),
                        )
                    y2 = spool.tile([P, 128], F32, tag="y2")
                    nc.scalar.copy(y2[:], py2[:])
                    nc.sync.dma_start(y_d[ms, 768:896], y2[:])
                    py3 = psum.tile([P, 128], F32, tag="pg", name="py3")
                    for it in range(IT):
                        nc.tensor.matmul(
                            py3[:], h_sl(it, mt), wd_sl(it, 896, 1024),
                            start=(it == 0), stop=(it == IT - 1),
                        )
                    # final exposed work: [128,128] copy on the idle DVE,
                    # DMA issued by sync which has been parked on the wait
                    y3 = spool.tile([P, 128], F32, tag="y3")
                    nc.vector.tensor_copy(y3[:], py3[:])
                    nc.sync.dma_start(y_d[ms, 896:1024], y3[:])

    nc.compile()
    return nc


def _get_compiled():
    global _compiled
    if _compiled is None:
        _compiled = _build()
    return _compiled


def _numpy_fallback(hidden_states, gate_kernel, up_kernel, down_kernel, group_sizes):
    # Exact reference math on host; only used for unexpected group_sizes.
    out = np.empty((hidden_states.shape[0], down_kernel.shape[2]), np.float32)
    start = 0
    for e in range(gate_kernel.shape[0]):
        g = int(group_sizes[e])
        x = hidden_states[start:start + g]
        gate = x @ gate_kernel[e]
        up = x @ up_kernel[e]
        sig = np.where(
            gate >= 0,
            1.0 / (1.0 + np.exp(-np.clip(gate, 0, None))),
            np.exp(np.clip(gate, None, 0))
            / (1.0 + np.exp(np.clip(gate, None, 0))),
        )
        h = gate * sig * up
        out[start:start + g] = h @ down_kernel[e]
        start += g
    out[start:] = 0.0
    return out


def _pack_h_major(w, c0, c1):
    # w: [H, C] slice cols [c0:c1] -> [P, KT*(c1-c0)] with layout
    # [p][ko*(c1-c0) + c]: per-partition data contiguous in DRAM.
    cw = c1 - c0
    return np.ascontiguousarray(
        w[:, c0:c1].reshape(KT, P, cw).transpose(1, 0, 2).reshape(P, KT * cw)
    )


def _make_in_maps(hidden_states, gate_kernel, up_kernel, down_kernel):
    bf = ml_dtypes.bfloat16
    in_maps = []
    for e in range(E):
        x_e = hidden_states[e * T:(e + 1) * T]
        # xp[p][ko*T + t] = x_e[t, ko*128+p]
        xp = np.ascontiguousarray(
            x_e.T.reshape(KT, P, T).transpose(1, 0, 2).reshape(P, KT * T)
        )
        wg = gate_kernel[e]
        wu = up_kernel[e]
        wd = down_kernel[e]
        m = {
            "xp": xp.astype(bf),
            "wg0": _pack_h_major(wg, 0, 128).astype(bf),
            "wg1": _pack_h_major(wg, 128, 256).astype(bf),
            "wg23": _pack_h_major(wg, 256, 512).astype(bf),
            "wu0": _pack_h_major(wu, 0, 128).astype(bf),
            "wu1": _pack_h_major(wu, 128, 256).astype(bf),
            "wu23": _pack_h_major(wu, 256, 512).astype(bf),
        }
        for b in range(1, WB):
            m[f"wgb{b}"] = _pack_h_major(wg, b * 512, (b + 1) * 512).astype(bf)
            m[f"wub{b}"] = _pack_h_major(wu, b * 512, (b + 1) * 512).astype(bf)
        # wd: [I, H]; halves over io: [p][io*H + h]
        m["wd0"] = np.ascontiguousarray(
            wd[0:1024].reshape(8, P, H).transpose(1, 0, 2).reshape(P, 8 * H)
        ).astype(bf)
        m["wd1"] = np.ascontiguousarray(
            wd[1024:2048].reshape(8, P, H).transpose(1, 0, 2).reshape(P, 8 * H)
        ).astype(bf)
        in_maps.append(m)
    return in_maps


def profile_run(inputs, tmpdir=None):
    """Dev helper (not used by grading): run with NTFF tracing, return exec ns."""
    nc = _get_compiled()
    in_maps = _make_in_maps(
        np.asarray(inputs["hidden_states"], np.float32),
        np.asarray(inputs["gate_kernel"], np.float32),
        np.asarray(inputs["up_kernel"], np.float32),
        np.asarray(inputs["down_kernel"], np.float32),
    )
    res = run_bass_kernel_spmd(
        nc, in_maps, core_ids=list(range(E)), trace=True, tmpdir=tmpdir
    )
    return res.exec_time_ns


def kernel(hidden_states, gate_kernel, up_kernel, down_kernel, group_sizes):
    hidden_states = np.asarray(hidden_states, dtype=np.float32)
    gate_kernel = np.asarray(gate_kernel, dtype=np.float32)
    up_kernel = np.asarray(up_kernel, dtype=np.float32)
    down_kernel = np.asarray(down_kernel, dtype=np.float32)
    gs = np.asarray(group_sizes)

    if not (gs.shape == (E,) and np.all(gs == T)):
        return _numpy_fallback(
            hidden_states, gate_kernel, up_kernel, down_kernel, gs
        )

    nc = _get_compiled()
    in_maps = _make_in_maps(hidden_states, gate_kernel, up_kernel, down_kernel)
    res = run_bass_kernel_spmd(nc, in_maps, core_ids=list(range(E)))
    return np.concatenate([res.results[e]["y"] for e in range(E)], axis=0)


# revision 16
# speedup vs baseline: 1.0428x; 1.0428x over previous
"""Trainium2 Bass kernel for Llama4TextExperts-style grouped MoE FFN (SwiGLU).

Full-input contract: kernel(**inputs) takes the complete unsharded tensors and
returns the full [4096, 1024] output. Internally: expert-parallel across the 8
NeuronCores — core e gets expert e's three weight matrices and that expert's
512-token group (tokens arrive pre-sorted by expert with equal group sizes).
All routing / transposition is done host-side in numpy; no collectives needed.

Per-core device program (three GEMMs, ~6.4 GFLOP):
  phase 1: gate^T = Wg^T-stationary @ x^T, up^T likewise; SwiGLU fused on
           ACT (Silu) + DVE (mul) into h^T [I, T] bf16 resident in SBUF.
  phase 2: y = h @ Wd with h^T slices stationary, Wd streaming; y lands
           untransposed in PSUM.

All DRAM parameters are host-packed partition-major so every DMA moves
2-16KB contiguous runs per partition (vs 256B for a naive [H, I] layout):
the DMA pipe hits peak rate sooner, which sets how early the PE can leave
its HAM warm-up and enter the real matmul stream.
"""

import numpy as np
import ml_dtypes

import concourse.bass as bass
import concourse.mybir as mybir
import concourse.tile as tile
from concourse.tile import add_dep_helper
from concourse import bacc
from concourse.bass_utils import run_bass_kernel_spmd

# Problem shape (hardcoded per contract)
E = 8          # experts == cores
T = 512        # tokens per expert group
H = 1024       # hidden
I = 2048       # intermediate
P = 128        # partitions
KT = H // P    # 8  k-tiles over hidden
IT = I // P    # 16 i-tiles over intermediate
WB = 4         # i-blocks of 512 columns for gate/up weight streaming
MT = T // P    # 4  token tiles
N_WARM = 6     # HAM warm-up matmuls before the first real chain (512 wide)

BF16 = mybir.dt.bfloat16
F32 = mybir.dt.float32

_compiled = None  # cached across calls


def _build():
    nc = bacc.Bacc(None)
    # Host-packed inputs: every tensor is [128 partitions, N] with the
    # partition's data contiguous in DRAM.
    xp_d = nc.declare_dram_parameter("xp", [P, KT * T], BF16, isOutput=False)
    wg0_d = nc.declare_dram_parameter("wg0", [P, KT * 128], BF16, isOutput=False)
    wg1_d = nc.declare_dram_parameter("wg1", [P, KT * 128], BF16, isOutput=False)
    wg23_d = nc.declare_dram_parameter("wg23", [P, KT * 256], BF16, isOutput=False)
    wu0_d = nc.declare_dram_parameter("wu0", [P, KT * 128], BF16, isOutput=False)
    wu1_d = nc.declare_dram_parameter("wu1", [P, KT * 128], BF16, isOutput=False)
    wu23_d = nc.declare_dram_parameter("wu23", [P, KT * 256], BF16, isOutput=False)
    wgb_d = [
        nc.declare_dram_parameter(f"wgb{b}", [P, KT * 512], BF16, isOutput=False)
        for b in range(1, WB)
    ]
    wub_d = [
        nc.declare_dram_parameter(f"wub{b}", [P, KT * 512], BF16, isOutput=False)
        for b in range(1, WB)
    ]
    wd0_d = nc.declare_dram_parameter("wd0", [P, 8 * H], BF16, isOutput=False)
    wd1_d = nc.declare_dram_parameter("wd1", [P, 8 * H], BF16, isOutput=False)
    y_d = nc.declare_dram_parameter("y", [T, H], F32, isOutput=True)

    with tile.TileContext(nc) as tc:
        with (
            tc.tile_pool(name="xpool", bufs=1) as xpool,
            tc.tile_pool(name="wdpool", bufs=1) as wdpool,
            tc.tile_pool(name="hpool", bufs=1) as hpool,
            tc.tile_pool(name="wpool", bufs=3) as wpool,
            tc.tile_pool(name="spool", bufs=3) as spool,
            tc.tile_pool(name="psum", bufs=2, space="PSUM") as psum,
        ):
            # --- HAM warm-up -------------------------------------------------
            # The PE clock sits at 1.2GHz until ~3.4us of sustained busy time,
            # and RE-throttles if an activity window (~3.4us) passes mostly
            # idle. Warm-up matmuls run before the first weights land, and
            # fill_mm() dummies are woven between the first real chains so a
            # late DMA never leaves the PE idle long enough to re-throttle.
            # All dummies form one accumulation chain into a PSUM bank that
            # the down-projection won't need until much later.
            warm = xpool.tile([P, 512], BF16, tag="warm", name="warm")
            nc.gpsimd.memset(warm[:], 0.0)
            pwarm = psum.tile([P, 512], F32, tag="py0", name="pwarm")
            fill_state = {"n": 0}

            def fill_mm(n):
                for _ in range(n):
                    nc.tensor.matmul(
                        pwarm[:], warm[:, 0:128], warm[:],
                        start=(fill_state["n"] == 0), stop=False,
                    )
                    fill_state["n"] += 1

            def fill_done():
                nc.tensor.matmul(
                    pwarm[:], warm[:, 0:128], warm[:],
                    start=False, stop=True,
                )

            fill_mm(N_WARM)

            # --- DMA issue schedule -----------------------------------------
            # Four engines issue in parallel (one instruction ~0.7us each);
            # each engine's queue is FIFO so per-engine order = priority.
            # Wave 1 (ungated): the critical set for the first it-blocks —
            # x (reused by every chain) + it0/it1 gate/up weights.
            # Later stages are gated on earlier DMA completions (lookahead
            # gating) so they don't steal round-robin bandwidth early.
            xt = xpool.tile([P, KT * T], BF16, tag="x", name="xt")
            wg_it0 = xpool.tile([P, KT * 128], BF16, tag="wg_it0")
            wg_it1 = xpool.tile([P, KT * 128], BF16, tag="wg_it1")
            wu_it0 = xpool.tile([P, KT * 128], BF16, tag="wu_it0")
            wu_it1 = xpool.tile([P, KT * 128], BF16, tag="wu_it1")
            wg_b0r = xpool.tile([P, KT * 256], BF16, tag="wg_b0r")
            wu_b0r = xpool.tile([P, KT * 256], BF16, tag="wu_b0r")
            hT_sb = hpool.tile([P, IT * T], BF16)
            wd_sb = wdpool.tile([P, IT * H], BF16)

            # ALL input DMAs ride sync's single HWDGE queue, FIFO-ordered
            # exactly by consumption order. With no other queue competing,
            # the head item gets the full (ramping) DMA bandwidth, so
            # delivery tracks the consumption schedule as closely as the
            # hardware allows. The early DMA supply curve is the binding
            # constraint on stream start; any out-of-order delivery shifts
            # a needed tensor behind bytes that could have waited.
            d0g = nc.sync.dma_start(wg_it0[:], wg0_d[:])
            nc.sync.dma_start(xt[:, 0:2 * T], xp_d[:, 0:2 * T])
            nc.sync.dma_start(xt[:, 2 * T:4 * T], xp_d[:, 2 * T:4 * T])
            nc.sync.dma_start(xt[:, 4 * T:6 * T], xp_d[:, 4 * T:6 * T])
            nc.sync.dma_start(xt[:, 6 * T:8 * T], xp_d[:, 6 * T:8 * T])
            d0u = nc.sync.dma_start(wu_it0[:], wu0_d[:])
            nc.sync.dma_start(wg_it1[:], wg1_d[:])
            d_A = nc.sync.dma_start(wu_it1[:], wu1_d[:])

            def gated(dma_call, dep):
                add_dep_helper(dma_call.ins, dep.ins, reason="dma staging throttle")
                return dma_call

            # it2/it3 of block 0, gated on the it0-g load (lookahead gating
            # keeps the pipe from draining dry between stages)
            gated(nc.sync.dma_start(wg_b0r[:], wg23_d[:]), d0g)
            d_B = gated(nc.sync.dma_start(wu_b0r[:], wu23_d[:]), d0g)
            # blocks 1..3 with two-stage lookahead gating
            wgf, wuf = {}, {}
            d_stage = [d_A, d_B]
            for wb in range(1, WB):
                wgb = wpool.tile([P, KT * 512], BF16, tag="wgf")
                gated(nc.sync.dma_start(wgb[:], wgb_d[wb - 1][:]),
                      d_stage[wb - 1])
                wgf[wb] = wgb
                wub = wpool.tile([P, KT * 512], BF16, tag="wuf")
                d_stage.append(
                    gated(nc.sync.dma_start(wub[:], wub_d[wb - 1][:]),
                          d_stage[wb - 1])
                )
                wuf[wb] = wub
            # Wd halves, chained behind the gate/up weight stream
            gated(nc.sync.dma_start(wd_sb[:, 0:8 * H], wd0_d[:]), d_stage[2])
            gated(nc.sync.dma_start(wd_sb[:, 8 * H:16 * H], wd1_d[:]), d_stage[3])

            def xk(kt):
                return xt[:, kt * T:(kt + 1) * T]

            def gu_slice(wb, itl, which):
                if wb == 0:
                    if itl < 2:
                        t = (wg_it0, wg_it1)[itl] if which == "g" else \
                            (wu_it0, wu_it1)[itl]
                        return lambda kt: t[:, kt * 128:(kt + 1) * 128]
                    t = wg_b0r if which == "g" else wu_b0r
                    base = (itl - 2) * 128
                    return lambda kt: t[:, kt * 256 + base:kt * 256 + base + 128]
                t = wgf[wb] if which == "g" else wuf[wb]
                base = itl * 128
                return lambda kt: t[:, kt * 512 + base:kt * 512 + base + 128]

            # --- gate/up + SwiGLU -------------------------------------------
            for wb in range(WB):
                for itl in range(4):
                    it = wb * 4 + itl
                    gsl = gu_slice(wb, itl, "g")
                    usl = gu_slice(wb, itl, "u")
                    pg = psum.tile([P, T], F32, tag="pg")
                    pu = psum.tile([P, T], F32, tag="pu")
                    for kt in range(KT):
                        nc.tensor.matmul(
                            pg[:], gsl(kt), xk(kt),
                            start=(kt == 0), stop=(kt == KT - 1),
                        )
                        # dummies bridge the gaps while x/weight DMAs are
                        # still ramping, keeping the activity monitor fed
                        if it == 0 and kt in (0, 2, 4, 6):
                            fill_mm(1)
                    if it == 0:
                        fill_mm(1)
                        fill_done()
                    for kt in range(KT):
                        nc.tensor.matmul(
                            pu[:], usl(kt), xk(kt),
                            start=(kt == 0), stop=(kt == KT - 1),
                        )
                    sg = spool.tile([P, T], F32)
                    nc.scalar.activation(
                        sg[:], pg[:], mybir.ActivationFunctionType.Silu
                    )
                    nc.vector.tensor_mul(
                        hT_sb[:, it * T:(it + 1) * T], sg[:], pu[:]
                    )

            # --- down projection --------------------------------------------
            def wd_sl(it, c0, c1):
                return wd_sb[:, it * H + c0:it * H + c1]

            def h_sl(it, ms):
                return hT_sb[:, it * T + ms * P:it * T + (ms + 1) * P]

            for mt in range(MT):
                ms = slice(mt * P, (mt + 1) * P)
                if mt < MT - 1:
                    py0 = psum.tile([P, 512], F32, tag="py0")
                    py1 = psum.tile([P, 512], F32, tag="py1")
                    for it in range(IT):
                        lhsT = h_sl(it, mt)
                        nc.tensor.matmul(
                            py0[:], lhsT, wd_sl(it, 0, 512),
                            start=(it == 0), stop=(it == IT - 1),
                        )
                        nc.tensor.matmul(
                            py1[:], lhsT, wd_sl(it, 512, 1024),
                            start=(it == 0), stop=(it == IT - 1),
                        )
                    y0 = spool.tile([P, 512], F32, tag="y0")
                    nc.scalar.copy(y0[:], py0[:])
                    nc.sync.dma_start(y_d[ms, 0:512], y0[:])
                    y1 = spool.tile([P, 512], F32, tag="y1")
                    nc.vector.tensor_copy(y1[:], py1[:])
                    nc.gpsimd.dma_start(y_d[ms, 512:1024], y1[:])
                else:
                    # Last token tile: run chains back-to-back (not
                    # interleaved) with shrinking widths 512/256/128/128 so
                    # after the very last matmul only a [128,128] copy + DMA
                    # is exposed, and earlier copies/DMAs overlap later
                    # matmul chains.
                    py0 = psum.tile([P, 512], F32, tag="py0")
                    for it in range(IT):
                        nc.tensor.matmul(
                            py0[:], h_sl(it, mt), wd_sl(it, 0, 512),
                            start=(it == 0), stop=(it == IT - 1),
                        )
                    y0 = spool.tile([P, 512], F32, tag="y0")
                    nc.scalar.copy(y0[:], py0[:])
                    nc.sync.dma_start(y_d[ms, 0:512], y0[:])
                    py1 = psum.tile([P, 256], F32, tag="py1")
                    for it in range(IT):
                        nc.tensor.matmul(
                            py1[:], h_sl(it, mt), wd_sl(it, 512, 768),
                            start=(it == 0), stop=(it == IT - 1),
                        )
                    y1 = spool.tile([P, 256], F32, tag="y1")
                    nc.vector.tensor_copy(y1[:], py1[:])
                    nc.gpsimd.dma_start(y_d[ms, 512:768], y1[:])
                    py2 = psum.tile([P, 128], F32, tag="pu", name="py2")
                    for it in range(IT):
                        nc.tensor.matmul(
                            py2[:], h_sl(it, mt), wd_sl(it, 768, 896),
                            start=(it == 0), stop=(it == IT - 1),
                        )
                    y2 = spool.tile([P, 128], F32, tag="y2")
                    nc.scalar.copy(y2[:], py2[:])
                    nc.sync.dma_start(y_d[ms, 768:896], y2[:])
                    py3 = psum.tile([P, 128], F32, tag="pg", name="py3")
                    for it in range(IT):
                        nc.tensor.matmul(
                            py3[:], h_sl(it, mt), wd_sl(it, 896, 1024),
                            start=(it == 0), stop=(it == IT - 1),
                        )
                    # final exposed work: [128,128] copy on the idle DVE,
                    # DMA issued by sync which has been parked on the wait
                    y3 = spool.tile([P, 128], F32, tag="y3")
                    nc.vector.tensor_copy(y3[:], py3[:])
                    nc.sync.dma_start(y_d[ms, 896:1024], y3[:])

    nc.compile()
    return nc


def _get_compiled():
    global _compiled
    if _compiled is None:
        _compiled = _build()
    return _compiled


def _numpy_fallback(hidden_states, gate_kernel, up_kernel, down_kernel, group_sizes):
    # Exact reference math on host; only used for unexpected group_sizes.
    out = np.empty((hidden_states.shape[0], down_kernel.shape[2]), np.float32)
    start = 0
    for e in range(gate_kernel.shape[0]):
        g = int(group_sizes[e])
        x = hidden_states[start:start + g]
        gate = x @ gate_kernel[e]
        up = x @ up_kernel[e]
        sig = np.where(
            gate >= 0,
            1.0 / (1.0 + np.exp(-np.clip(gate, 0, None))),
            np.exp(np.clip(gate, None, 0))
            / (1.0 + np.exp(np.clip(gate, None, 0))),
        )
        h = gate * sig * up
        out[start:start + g] = h @ down_kernel[e]
        start += g
    out[start:] = 0.0
    return out


def _pack_h_major(w, c0, c1):
    # w: [H, C] slice cols [c0:c1] -> [P, KT*(c1-c0)] with layout
    # [p][ko*(c1-c0) + c]: per-partition data contiguous in DRAM.
    cw = c1 - c0
    return np.ascontiguousarray(
        w[:, c0:c1].reshape(KT, P, cw).transpose(1, 0, 2).reshape(P, KT * cw)
    )


def _make_in_maps(hidden_states, gate_kernel, up_kernel, down_kernel):
    bf = ml_dtypes.bfloat16
    in_maps = []
    for e in range(E):
        x_e = hidden_states[e * T:(e + 1) * T]
        # xp[p][ko*T + t] = x_e[t, ko*128+p]
        xp = np.ascontiguousarray(
            x_e.T.reshape(KT, P, T).transpose(1, 0, 2).reshape(P, KT * T)
        )
        wg = gate_kernel[e]
        wu = up_kernel[e]
        wd = down_kernel[e]
        m = {
            "xp": xp.astype(bf),
            "wg0": _pack_h_major(wg, 0, 128).astype(bf),
            "wg1": _pack_h_major(wg, 128, 256).astype(bf),
            "wg23": _pack_h_major(wg, 256, 512).astype(bf),
            "wu0": _pack_h_major(wu, 0, 128).astype(bf),
            "wu1": _pack_h_major(wu, 128, 256).astype(bf),
            "wu23": _pack_h_major(wu, 256, 512).astype(bf),
        }
        for b in range(1, WB):
            m[f"wgb{b}"] = _pack_h_major(wg, b * 512, (b + 1) * 512).astype(bf)
            m[f"wub{b}"] = _pack_h_major(wu, b * 512, (b + 1) * 512).astype(bf)
        # wd: [I, H]; halves over io: [p][io*H + h]
        m["wd0"] = np.ascontiguousarray(
            wd[0:1024].reshape(8, P, H).transpose(1, 0, 2).reshape(P, 8 * H)
        ).astype(bf)
        m["wd1"] = np.ascontiguousarray(
            wd[1024:2048].reshape(8, P, H).transpose(1, 0, 2).reshape(P, 8 * H)
        ).astype(bf)
        in_maps.append(m)
    return in_maps


def profile_run(inputs, tmpdir=None):
    """Dev helper (not used by grading): run with NTFF tracing, return exec ns."""
    nc = _get_compiled()
    in_maps = _make_in_maps(
        np.asarray(inputs["hidden_states"], np.float32),
        np.asarray(inputs["gate_kernel"], np.float32),
        np.asarray(inputs["up_kernel"], np.float32),
        np.asarray(inputs["down_kernel"], np.float32),
    )
    res = run_bass_kernel_spmd(
        nc, in_maps, core_ids=list(range(E)), trace=True, tmpdir=tmpdir
    )
    return res.exec_time_ns


def kernel(hidden_states, gate_kernel, up_kernel, down_kernel, group_sizes):
    hidden_states = np.asarray(hidden_states, dtype=np.float32)
    gate_kernel = np.asarray(gate_kernel, dtype=np.float32)
    up_kernel = np.asarray(up_kernel, dtype=np.float32)
    down_kernel = np.asarray(down_kernel, dtype=np.float32)
    gs = np.asarray(group_sizes)

    if not (gs.shape == (E,) and np.all(gs == T)):
        return _numpy_fallback(
            hidden_states, gate_kernel, up_kernel, down_kernel, gs
        )

    nc = _get_compiled()
    in_maps = _make_in_maps(hidden_states, gate_kernel, up_kernel, down_kernel)
    res = run_bass_kernel_spmd(nc, in_maps, core_ids=list(range(E)))
    return np.concatenate([res.results[e]["y"] for e in range(E)], axis=0)
